# revision 8
# baseline (speedup 1.0000x reference)
"""CTRNN (Dale-constrained leaky RNN) Trainium2 kernel.

Math (per reference):
    Weff    = |Wahh| * mask
    xin_t   = x_t @ Wahx.T + bah
    ah_{t+1} = 0.9*ah_t + 0.1*(retanh(ah_t) @ Weff.T + xin_t)
    hs[t]   = retanh(ah_{t+1});   y = hs @ Wyh.T + by
    retanh(a) = max(tanh(a), 0) = tanh(max(a, 0))

Strategy: data-parallel over batch (B=64 -> 8 batches per NeuronCore); the
sequential T=1000 scan runs locally per core in neuron-major layout
[128 part, chunk, batch], with the recurrent weight as the PE stationary
operand (bf16, fp32 PSUM accumulate) so no per-step transposes are needed.
The input drive is precomputed on-device into HBM; the readout y is folded
into the scan as one small matmul block every U steps from an SBUF h-ring.
"""

import numpy as np
import ml_dtypes

import concourse.bass as bass
import concourse.bacc as bacc
import concourse.mybir as mybir
from concourse.tile import TileContext
from concourse.bass_utils import run_bass_kernel_spmd
from concourse.alu_op_type import AluOpType

F32 = mybir.dt.float32
BF16 = mybir.dt.bfloat16
AF = mybir.ActivationFunctionType

B, T, NI, N, NO = 64, 1000, 128, 1024, 64
NCORES = 8
BL = B // NCORES            # batches per core
MC = N // 128               # m-chunks (output neuron chunks)
KC = N // 128               # k-chunks (contraction chunks)
U = 50                      # timesteps per block
NB = T // U                 # blocks
DT = 0.1                    # dt/tau
DECAY = 1.0 - DT

TRACE = False               # set by test harness for profiling
LAST_RESULTS = None         # BassKernelResults of the last run


def _bcast_ap(t, shape_counts, steps):
    """Build an AP on tile t with explicit [step, count] pairs (after the
    partition dim, which is taken from t)."""
    ap = [t.ap[0]] + [[s, c] for s, c in zip(steps, shape_counts)]
    return bass.AP(tensor=t.tensor, offset=t.offset, ap=ap)


def _build_nc():
    nc = bacc.Bacc("TRN2", target_bir_lowering=False)

    x_d = nc.dram_tensor("x", [BL * T, NI], BF16, kind="ExternalInput")
    wq_d = nc.dram_tensor("wq", [128, KC, MC, 128], BF16, kind="ExternalInput")
    wx_d = nc.dram_tensor("wx", [NI, MC, 128], BF16, kind="ExternalInput")
    wy_d = nc.dram_tensor("wy", [128, KC, NO], BF16, kind="ExternalInput")
    bah_d = nc.dram_tensor("bah", [128, MC], F32, kind="ExternalInput")
    ah0_d = nc.dram_tensor("ah0", [128, MC], F32, kind="ExternalInput")
    by_d = nc.dram_tensor("by", [NO, 1], F32, kind="ExternalInput")
    y_d = nc.dram_tensor("y", [NO, T, BL], F32, kind="ExternalOutput")
    xin_d = nc.dram_tensor("xin", [NB, 128, MC, BL, U], F32, kind="Internal")

    with TileContext(nc) as tc:
        with tc.tile_pool(name="consts", bufs=1) as consts:
            wsta = consts.tile([128, KC, MC, 128], BF16)
            nc.sync.dma_start(wsta, wq_d[:])
            wx = consts.tile([NI, MC, 128], BF16)
            nc.sync.dma_start(wx, wx_d[:])
            wy = consts.tile([128, KC, NO], BF16)
            nc.sync.dma_start(wy, wy_d[:])
            bahT = consts.tile([128, MC], F32)
            nc.sync.dma_start(bahT, bah_d[:])
            ah0T = consts.tile([128, MC], F32)
            nc.sync.dma_start(ah0T, ah0_d[:])
            byv = consts.tile([NO, 1], F32)
            nc.sync.dma_start(byv, by_d[:])

            xT = consts.tile([NI, BL * T], BF16)        # x transposed, bf16
            yb = consts.tile([NO, NB, U, BL], F32)      # y accumulator
            ah = consts.tile([128, MC, BL], F32)        # recurrent state
            ring = consts.tile([128, U + 1, MC, BL], BF16)  # h ring (slot, chunk, b)

            # ---- Phase 1: transpose x -> xT via DMA xbar transpose ----
            nc.sync.dma_start_transpose(xT, x_d[:])

            # ---- Phase 2: xin = 0.1*(x @ Wahx.T + bah), blocked to HBM ----
            xTv = xT.rearrange("p (b t) -> p b t", t=T)
            with tc.tile_pool(name="ph2", bufs=3) as ph2, \
                 tc.tile_pool(name="ph2ps", bufs=2, space="PSUM") as ph2ps:
                for j in range(NB):
                    for mi in range(MC):
                        px = ph2ps.tile([128, BL, U], F32, tag="px")
                        nc.tensor.matmul(px, lhsT=wx[:, mi, :],
                                         rhs=xTv[:, :, j * U:(j + 1) * U],
                                         start=True, stop=True)
                        xs = ph2.tile([128, BL, U], F32, tag="xs")
                        nc.scalar.activation(xs, px, AF.Identity,
                                             bias=bahT[:, mi:mi + 1], scale=1.0)
                        nc.sync.dma_start(xin_d[j, :, mi, :, :], xs)

            # ---- Init: ah = broadcast(ah0), ring[0] = retanh(ah) ----
            with tc.tile_pool(name="initp", bufs=1) as initp:
                ah0b = _bcast_ap(ah0T, [MC, BL], [1, 0])
                nc.vector.tensor_copy(ah, ah0b)
                r0t = initp.tile([128, MC, BL], F32)
                nc.vector.tensor_scalar_max(r0t, ah, 0.0)
                nc.scalar.activation(ring[:, 0], r0t, AF.Tanh)

            # ---- Phase 3: the scan ----
            H = MC // 2
            with tc.tile_pool(name="scan", bufs=2) as scan_p, \
                 tc.tile_pool(name="scps", bufs=2, space="PSUM") as scps, \
                 tc.tile_pool(name="yps", bufs=2, space="PSUM") as yps:
                with tc.For_i(0, NB, 1,
                              hint_engines=(mybir.EngineType.PE,)) as j:
                    xin_blk = scan_p.tile([128, MC, BL, U], F32, tag="xinb")
                    nc.sync.dma_start(xin_blk, xin_d[bass.ds(j, 1), :, :, :, :])
                    for th in range(U):
                        s_r = th
                        s_w = th + 1
                        u_t = scan_p.tile([128, MC, BL], F32, tag="u")
                        nc.vector.scalar_tensor_tensor(
                            out=u_t, in0=ah, scalar=DECAY,
                            in1=xin_blk[:, :, :, th],
                            op0=AluOpType.mult, op1=AluOpType.add)
                        for half in range(2):
                            ps = scps.tile([128, H, BL], F32,
                                           tag=f"ps{half}")
                            for mloc in range(H):
                                mi = half * H + mloc
                                for ki in range(KC):
                                    nc.tensor.matmul(
                                        ps[:, mloc, :],
                                        lhsT=wsta[:, ki, mi, :],
                                        rhs=ring[:, s_r, ki, :],
                                        start=(ki == 0), stop=(ki == KC - 1))
                            sl = slice(half * H, half * H + H)
                            nc.vector.tensor_tensor(
                                out=ah[:, sl, :], in0=ps, in1=u_t[:, sl, :],
                                op=AluOpType.add)
                            rr = scan_p.tile([128, H, BL], F32, tag="rr")
                            nc.vector.tensor_scalar_max(rr, ah[:, sl, :], 0.0)
                            nc.scalar.activation(ring[:, s_w, sl, :], rr,
                                                 AF.Tanh)
                    # carry last h into slot 0 for the next block
                    nc.vector.tensor_copy(ring[:, 0], ring[:, U])
                    # y block: y[o, t, b] over this block's 50 steps
                    yp = yps.tile([NO, U, BL], F32, tag="yp")
                    for ki in range(KC):
                        nc.tensor.matmul(yp, lhsT=wy[:, ki, :],
                                         rhs=ring[:, 1:U + 1, ki, :],
                                         start=(ki == 0), stop=(ki == KC - 1))
                    ybv = yb.rearrange("o n u b -> o n (u b)")
                    ypv = yp.rearrange("o u b -> o (u b)")
                    nc.vector.tensor_copy(
                        ybv[:, bass.ds(j, 1), :],
                        _bcast_ap(ypv, [1, U * BL], [0, 1]))

            # ---- Post: add by, write y out ----
            ybf = yb.rearrange("o n u b -> o (n u b)")
            nc.scalar.activation(ybf, ybf, AF.Identity, bias=byv[:, 0:1],
                                 scale=1.0)
            nc.sync.dma_start(y_d[:], yb.rearrange("o n u b -> o (n u) b"))

    nc.compile()
    return nc


_NC_CACHE = {}


def _get_nc():
    if "nc" not in _NC_CACHE:
        _NC_CACHE["nc"] = _build_nc()
    return _NC_CACHE["nc"]


def prepare_in_maps(x, Wahx, Wahh, Wyh, bah, by, ah0, mask):
    bf16 = ml_dtypes.bfloat16
    x = np.asarray(x, np.float32)
    Wahx = np.asarray(Wahx, np.float32)
    Wahh = np.asarray(Wahh, np.float32)
    Wyh = np.asarray(Wyh, np.float32)
    bah = np.asarray(bah, np.float32)
    by = np.asarray(by, np.float32)
    ah0 = np.asarray(ah0, np.float32)
    mask = np.asarray(mask, np.float32)

    weff = np.abs(Wahh) * mask                       # [m, k]
    wq = (DT * weff).reshape(MC, 128, KC, 128)       # [mi, mm, ki, kk]
    wq_l = np.ascontiguousarray(wq.transpose(3, 2, 0, 1)).astype(bf16)
    wx_l = np.ascontiguousarray(
        (DT * Wahx).T.reshape(NI, MC, 128)).astype(bf16)
    wy_l = np.ascontiguousarray(
        Wyh.T.reshape(KC, 128, NO).transpose(1, 0, 2)).astype(bf16)
    bah_l = np.ascontiguousarray((DT * bah).reshape(MC, 128).T,
                                 dtype=np.float32)
    ah0_l = np.ascontiguousarray(ah0.reshape(MC, 128).T, dtype=np.float32)
    by_l = np.ascontiguousarray(by.reshape(NO, 1), dtype=np.float32)

    x16 = x.reshape(B, T * NI).astype(bf16)
    in_maps = []
    for c in range(NCORES):
        xc = np.ascontiguousarray(
            x16[c * BL:(c + 1) * BL].reshape(BL * T, NI))
        in_maps.append(dict(x=xc, wq=wq_l, wx=wx_l, wy=wy_l, bah=bah_l,
                            ah0=ah0_l, by=by_l))
    return in_maps


def kernel(x, Wahx, Wahh, Wyh, bah, by, ah0, mask):
    global LAST_RESULTS
    in_maps = prepare_in_maps(x, Wahx, Wahh, Wyh, bah, by, ah0, mask)
    nc = _get_nc()
    res = run_bass_kernel_spmd(nc, in_maps, core_ids=list(range(NCORES)),
                               trace=TRACE)
    LAST_RESULTS = res

    out = np.empty((B, T, NO), np.float32)
    for c in range(NCORES):
        yc = np.asarray(res.results[c]["y"], np.float32)   # [NO, T, BL]
        out[c * BL:(c + 1) * BL] = yc.transpose(2, 1, 0)
    return out
